# revision 1
# baseline (speedup 1.0000x reference)
"""Trainium2 Bass kernel v2 for nn_Attention_14190571946482.

Causal self-attention (diagonal masked too), scores computed TRANSPOSED:
  scoreT[kk, q] = (kT chunk as weights) @ qT          (kk on partitions)
  attnT = exp(scoreT * scale) -> bf16 SBUF            (frontier masked)
  ctx[q, u] += (attnT block as weights) @ v_chunk
  den[q]    += (attnT block as weights) @ ones        (N=1 matmul)
No PE transposes, no attn psum->sbuf copies; den lands per-partition so
normalize is a per-partition ACT scale.

Sharding: 8 cores = 4 batches x 2 roles; role r owns tiles {2j+r}.
Per core 2 groups of 4 slots: G2 = tiles {8..15} (chunks 0..15) first,
then G1 = tiles {0..7} (chunks 0..7). Slot j holds tile pair
(tbase+2j, +1); role-dependent structure is carried in input data
(qx gather, mask blocks, misc columns) so the program is SPMD-identical.
Per chunk c only the live suffix of slots j >= j0 is computed; one
[128,128] multiplicative mask block per frontier chunk (content per
role: tri / ones / zeros). Row 0 (fully masked) is blended to mean(v)
on the psum before normalize via host-provided selectors.
"""

import sys

sys.path.insert(0, "/opt/trn_rl_repo")

import numpy as np
import ml_dtypes

import concourse.bass as bass
import concourse.bacc as bacc
import concourse.mybir as mybir
from concourse.tile import TileContext
from concourse import bass_utils

# Note: walrus's --enable-ldw-opt pass was tried and rejects this kernel
# ("InstLdweights is not compatible with LDW optimization") — leave it off.

BF16 = ml_dtypes.bfloat16

B, S, D, U = 4, 2048, 512, 512
P = 128
SCALE = 1.0 / float(np.sqrt(np.float32(D)))
GROUPS = [(8, 16), (0, 8)]  # (tbase, nchunks): G2 first, then G1
NSLOT = 8                   # output blocks: b=0..3 G2 slots, 4..7 G1
SPECIAL = 4                 # G1 slot 0 holds tiles (0,1): row-0 blend
NWARM = 8

_nc_cache = None


def build_nc():
    global _nc_cache
    if _nc_cache is not None:
        return _nc_cache

    f32 = mybir.dt.float32
    bf16 = mybir.dt.bfloat16

    nc = bacc.Bacc()
    xT_d = nc.declare_dram_parameter("xT", [D, S], bf16, isOutput=False)
    qx_d = nc.declare_dram_parameter("qx", [D, NSLOT * P], bf16, isOutput=False)
    wq_d = nc.declare_dram_parameter("wq", [D, U], bf16, isOutput=False)
    wv_d = nc.declare_dram_parameter("wv", [D, U], bf16, isOutput=False)
    wk_d = nc.declare_dram_parameter("wk", [D, U], bf16, isOutput=False)
    # 16 frontier mask blocks [128,128]: G2 chunks 8..15, then G1 0..7.
    mm_d = nc.declare_dram_parameter("maskblk", [P, 16 * P], bf16, isOutput=False)
    # misc f32: [0,0] rsel0 (row-0 ctx factor), [0,1] rscale (1/S or 0),
    # cols 8..15: per-output-block sume column.
    ms_d = nc.declare_dram_parameter("misc", [P, 16], f32, isOutput=False)
    out_d = nc.declare_dram_parameter("out", [NSLOT * P, U], bf16, isOutput=True)

    with TileContext(nc) as tc:
        with (
            tc.tile_pool(name="cst", bufs=1) as cst,
            tc.tile_pool(name="work", bufs=4) as work,
            tc.tile_pool(name="small", bufs=8) as small,
            tc.tile_pool(name="psA", bufs=3, space="PSUM") as psA,
            tc.tile_pool(name="psC", bufs=4, space="PSUM") as psC,
            tc.tile_pool(name="psD", bufs=1, space="PSUM") as psD,
        ):
            # ---- on-chip constants ----
            wu = cst.tile([P, 512], bf16, tag="wu")
            nc.vector.memset(wu, 0.0)
            ones_c = cst.tile([P, 1], bf16, tag="ones")
            nc.gpsimd.memset(ones_c, 1.0)

            # ---- input DMAs: 4 issue queues in parallel (each DMA queue
            # caps ~180 GB/s); xT0 + wv lead on separate queues so the kT
            # g-loop can start ~11us in ----
            xT_t = cst.tile([P, 4, S], bf16, tag="xT")
            xT_r = xT_d.rearrange("(d p) s -> p d s", p=P)
            wv_t = cst.tile([P, 4, U], bf16, tag="wv")
            wk_t = cst.tile([P, 4, U], bf16, tag="wk")
            wq_t = cst.tile([P, 4, U], bf16, tag="wq")
            qx_t = cst.tile([P, 4, NSLOT * P], bf16, tag="qx")
            maskblk = cst.tile([P, 16 * P], bf16, tag="maskblk")
            misc = cst.tile([P, 16], f32, tag="misc")
            # Arrival-vs-consumer margins are clock-critical: a >~3us PE gap
            # resets the p-state (measured +18us slowdown). All xT slices
            # precede the late-needed weights; ~4us of early-phase DMA wait
            # is conserved under any reorder (2 queues x ~1.5MB early bytes),
            # so tolerate the small v-vs-wk wait rather than risk a reset.
            nc.sync.dma_start(out=xT_t[:, :, 0:512], in_=xT_r[:, :, 0:512])
            nc.scalar.dma_start(out=wv_t, in_=wv_d.rearrange("(d p) u -> p d u", p=P))
            nc.sync.dma_start(out=xT_t[:, :, 512:1024], in_=xT_r[:, :, 512:1024])
            nc.scalar.dma_start(out=xT_t[:, :, 1024:1536],
                                in_=xT_r[:, :, 1024:1536])
            nc.sync.dma_start(out=xT_t[:, :, 1536:2048],
                              in_=xT_r[:, :, 1536:2048])
            nc.scalar.dma_start(out=wk_t, in_=wk_d.rearrange("(d p) u -> p d u", p=P))
            nc.scalar.dma_start(out=qx_t, in_=qx_d.rearrange("(d p) s -> p d s", p=P))
            nc.sync.dma_start(out=wq_t, in_=wq_d.rearrange("(d p) u -> p d u", p=P))
            nc.sync.dma_start(out=maskblk, in_=mm_d[:, :])
            nc.sync.dma_start(out=misc, in_=ms_d[:, :])

            wq = [wq_t[:, d, :] for d in range(4)]
            qx = [qx_t[:, d, :] for d in range(4)]
            xT = [xT_t[:, d, :] for d in range(4)]
            wv = [wv_t[:, d, :] for d in range(4)]
            wk = [wk_t[:, d, :] for d in range(4)]

            # ---- PE warm-up: ramp the HAM clock while DMAs land ----
            for _ in range(NWARM):
                wups = psA.tile([P, 512], f32, tag="blk")
                nc.tensor.matmul(wups, lhsT=wu[:, :P], rhs=wu,
                                 start=True, stop=True)

            # ---- phase 1: kT [u, s], v [s, u], qT [u, 1024] ----
            # kT and v interleaved per xT g-slice: compute fills the wait
            # for the next 512-col xT DMA slice.
            kT = [cst.tile([P, S], bf16, tag=f"kT{u}", name=f"kT{u}")
                  for u in range(4)]
            v_sb = [cst.tile([P, U], bf16, tag=f"v{sc}", name=f"v{sc}")
                    for sc in range(16)]
            ci = 0
            for g in range(4):
                for u in range(4):
                    ps = psA.tile([P, 512], f32, tag="blk")
                    for d in range(4):
                        nc.tensor.matmul(
                            ps,
                            lhsT=wv[d][:, u * P:(u + 1) * P],
                            rhs=xT[d][:, g * 512:(g + 1) * 512],
                            start=(d == 0), stop=(d == 3),
                        )
                    dst = kT[u][:, g * 512:(g + 1) * 512]
                    if ci % 2 == 0:
                        nc.vector.tensor_copy(dst, ps)
                    else:
                        nc.scalar.copy(dst, ps)
                    ci += 1
                for sc in range(4 * g, 4 * g + 4):
                    ps = psA.tile([P, 512], f32, tag="blk")
                    for d in range(4):
                        nc.tensor.matmul(
                            ps,
                            lhsT=xT[d][:, sc * P:(sc + 1) * P],
                            rhs=wk[d],
                            start=(d == 0), stop=(d == 3),
                        )
                    if sc % 2 == 0:
                        nc.scalar.copy(v_sb[sc], ps)
                    else:
                        nc.vector.tensor_copy(v_sb[sc], ps)

            qT = [cst.tile([P, NSLOT * P], bf16, tag=f"qT{u}", name=f"qT{u}")
                  for u in range(4)]
            for u in range(4):
                for h in range(2):
                    ps = psA.tile([P, 512], f32, tag="blk")
                    for d in range(4):
                        nc.tensor.matmul(
                            ps,
                            lhsT=wq[d][:, u * P:(u + 1) * P],
                            rhs=qx[d][:, h * 512:(h + 1) * 512],
                            start=(d == 0), stop=(d == 3),
                        )
                    dst = qT[u][:, h * 512:(h + 1) * 512]
                    if (u + h) % 2 == 0:
                        nc.scalar.copy(dst, ps)
                    else:
                        nc.vector.tensor_copy(dst, ps)

            # ---- mean-of-v (for the fully-masked global row 0) ----
            xs16 = []
            for d in range(4):
                xs = small.tile([P, 1], f32, tag="xs")
                nc.vector.reduce_sum(xs, xT[d], axis=mybir.AxisListType.X)
                x16 = small.tile([P, 1], bf16, tag="xs16")
                nc.vector.tensor_copy(x16, xs)
                xs16.append(x16)
            vm_ps = psA.tile([1, 512], f32, tag="blk")
            for d in range(4):
                nc.tensor.matmul(vm_ps, lhsT=xs16[d], rhs=wk[d],
                                 start=(d == 0), stop=(d == 3))
            vm_sb = cst.tile([1, 512], f32, tag="vm_sb")
            nc.vector.tensor_scalar_mul(vm_sb, vm_ps, misc[0:1, 1:2])

            # ---- phase 2: transposed-score attention ----
            # one psum bank holds all 8 slots' denominators (col b = 4g+j);
            # pre-zero it and accumulate with start=False throughout so no
            # start=True write can clobber a neighboring column's running sum
            den_t = psD.tile([P, 8], f32, tag="dent", name="dent")
            nc.vector.memset(den_t, 0.0)
            for g, (tbase, nchunks) in enumerate(GROUPS):
                ctx_ps = [psC.tile([P, 512], f32, tag="ctx", name=f"ctx{g}_{j}")
                          for j in range(4)]
                mask_base = 0 if g == 0 else 8
                for c in range(nchunks):
                    j0 = max(0, (c - tbase) // 2)
                    ncols = (4 - j0) * P
                    qoff = g * 512 + j0 * P
                    sc_ps = psA.tile([P, 512], f32, tag="blk")
                    for u in range(4):
                        nc.tensor.matmul(
                            sc_ps[:, :ncols],
                            lhsT=kT[u][:, c * P:(c + 1) * P],
                            rhs=qT[u][:, qoff:qoff + ncols],
                            start=(u == 0), stop=(u == 3),
                        )
                    attnT = work.tile([P, 512], bf16, tag="attnT")
                    nc.scalar.activation(
                        attnT[:, :ncols], sc_ps[:, :ncols],
                        mybir.ActivationFunctionType.Exp, scale=SCALE,
                    )
                    cl = c - tbase
                    if cl >= 0:
                        j = cl // 2
                        mb = (mask_base + cl) * P
                        sl = attnT[:, (j - j0) * P:(j - j0 + 1) * P]
                        nc.vector.tensor_mul(sl, sl, maskblk[:, mb:mb + P])
                    for j in range(j0, 4):
                        last = (cl == 2 * j + 1)
                        b = 4 * g + j
                        blk = attnT[:, (j - j0) * P:(j - j0 + 1) * P]
                        if last:
                            # den leads the critical rcp -> normalize path
                            nc.tensor.matmul(den_t[:, b:b + 1], lhsT=blk,
                                             rhs=ones_c, start=False,
                                             stop=True, skip_group_check=True)
                            nc.tensor.matmul(ctx_ps[j], lhsT=blk,
                                             rhs=v_sb[c], start=(c == 0),
                                             stop=True)
                            den = small.tile([P, 1], f32, tag="den")
                            nc.vector.tensor_add(den, den_t[:, b:b + 1],
                                                 misc[:, 8 + b:9 + b])
                            rcp = small.tile([P, 1], f32, tag="rcp")
                            nc.vector.reciprocal(rcp, den)
                            if b == SPECIAL:
                                # row 0 of role 0 = mean(v): on psum f32
                                nc.vector.tensor_scalar_mul(
                                    ctx_ps[j][0:1, :], ctx_ps[j][0:1, :],
                                    misc[0:1, 0:1])
                                nc.vector.tensor_add(
                                    ctx_ps[j][0:1, :], ctx_ps[j][0:1, :],
                                    vm_sb)
                            ctx_sb = work.tile([P, 512], bf16, tag="ctxs")
                            for hh in range(2):
                                nc.scalar.activation(
                                    ctx_sb[:, hh * 256:(hh + 1) * 256],
                                    ctx_ps[j][:, hh * 256:(hh + 1) * 256],
                                    mybir.ActivationFunctionType.Copy,
                                    scale=rcp)
                                nc.sync.dma_start(
                                    out=out_d[b * P:(b + 1) * P,
                                              hh * 256:(hh + 1) * 256],
                                    in_=ctx_sb[:, hh * 256:(hh + 1) * 256])
                        else:
                            nc.tensor.matmul(ctx_ps[j], lhsT=blk,
                                             rhs=v_sb[c], start=(c == 0),
                                             stop=False)
                            nc.tensor.matmul(den_t[:, b:b + 1], lhsT=blk,
                                             rhs=ones_c, start=False,
                                             stop=False,
                                             skip_group_check=True)

    nc.compile()
    _nc_cache = nc
    return nc


def tile_of_block(b, r):
    """Global q-tile held by output block b on role r."""
    return (8 + 2 * b + r) if b < 4 else (2 * (b - 4) + r)


def host_inputs(query, Wq, Wv, Wk):
    """Build per-core input maps. query [B,S,D] f32; W* [D,U] f32."""
    wq16 = Wq.astype(BF16)
    wv16 = Wv.astype(BF16)
    wk16 = Wk.astype(BF16)

    p = np.arange(P)[:, None]   # kk within chunk
    f = np.arange(P)[None, :]   # q within tile
    tri = (p < f).astype(np.float32)        # diag block: kk < q valid
    ones_b = np.ones((P, P), np.float32)
    zeros_b = np.zeros((P, P), np.float32)

    masks = {}
    for r in range(2):
        blocks = []
        for g, (tbase, nchunks) in enumerate(GROUPS):
            for cl in range(8):
                # chunk c = tbase + cl, affected slot j = cl//2,
                # role tile t = tbase + 2*(cl//2) + r
                c = tbase + cl
                t = tbase + 2 * (cl // 2) + r
                if c < t:
                    blocks.append(ones_b)
                elif c == t:
                    blocks.append(tri)
                else:
                    blocks.append(zeros_b)
        masks[r] = np.concatenate(blocks, axis=1).astype(BF16)

    in_maps = []
    for core in range(8):
        b_, r = core // 2, core % 2
        xTb = np.ascontiguousarray(query[b_].T).astype(BF16)      # [D, S]
        cols = np.concatenate(
            [np.arange(P * tile_of_block(b, r), P * tile_of_block(b, r) + P)
             for b in range(NSLOT)]
        )
        qx = np.ascontiguousarray(xTb[:, cols])                   # [D, 1024]
        misc = np.zeros((P, 16), np.float32)
        misc[0, 0] = 0.0 if r == 0 else 1.0      # rsel0
        misc[0, 1] = (1.0 / S) if r == 0 else 0.0  # rscale
        if r == 0:
            misc[0, 8 + SPECIAL] = 1.0           # den fix for global row 0
        in_maps.append({
            "xT": xTb, "qx": qx,
            "wq": wq16, "wv": wv16, "wk": wk16,
            "maskblk": masks[r], "misc": misc,
        })
    return in_maps


def assemble_output(results):
    """results: list of 8 dicts with 'out' [1024, 512] bf16."""
    out = np.zeros((B, S, U), np.float32)
    for core in range(8):
        b_, r = core // 2, core % 2
        o = np.asarray(results[core]["out"], dtype=np.float32)
        for b in range(NSLOT):
            t = tile_of_block(b, r)
            out[b_, P * t:P * (t + 1), :] = o[P * b:P * (b + 1), :]
    return out


def run(query, Wq, Wv, Wk, **kwargs):
    nc = build_nc()
    in_maps = host_inputs(
        np.asarray(query, np.float32), np.asarray(Wq, np.float32),
        np.asarray(Wv, np.float32), np.asarray(Wk, np.float32),
    )
    res = bass_utils.run_bass_kernel_spmd(nc, in_maps, list(range(8)), **kwargs)
    return assemble_output(res.results), res


def kernel(query, Wq, Wv, Wk):
    out, _ = run(query, Wq, Wv, Wk)
    return out


if __name__ == "__main__":
    rng = np.random.default_rng(0)
    q = rng.standard_normal((B, S, D), dtype=np.float32)
    scale = np.sqrt(2.0 / (D + U)).astype(np.float32)
    Wq = rng.standard_normal((D, U), dtype=np.float32) * scale
    Wv = rng.standard_normal((D, U), dtype=np.float32) * scale
    Wk = rng.standard_normal((D, U), dtype=np.float32) * scale
    out = kernel(q, Wq, Wv, Wk)
    print(out.shape, out.dtype, np.abs(out).mean())

